# revision 1
# baseline (speedup 1.0000x reference)
"""Chamfer distance kernel for 8 TRN2 NeuronCores (Bass/Tile).

Problem: points1 [16384, 3], points2 [16384, 3] fp32.
  out = sum_i min_j ||p1_i - p2_j|| + sum_j min_i ||p1_i - p2_j||   (scalar)

Strategy
--------
sqrt is monotonic, so min_j ||.|| = sqrt(min_j ||.||^2): only squared
distances are ever materialized, and sqrt runs on the host over the 32K
row-minima.

Squared distances come from a single K=5 matmul with augmented points:
    a_i = [x, y, z, |p|^2, 1]          (lhsT form, stationary)
    b_j = [-2x, -2y, -2z, 1, |p|^2]    (rhs form, moving)
    a_i . b_j = |p1_i|^2 + |p2_j|^2 - 2 p1_i.p2_j = d2(i, j)

Both chamfer terms are row-min problems (term2 is a row-min of the
transposed matrix = distance matrix with roles swapped), so each core
computes row-minima for 2048 rows of D(p1_chunk, p2) and 2048 rows of
D(p2_chunk, p1): 32 row-tiles of 128 rows. Row-min over the free axis is a
native DVE tensor_reduce straight out of PSUM - no partition-axis
reductions, no collectives.

Windowed (KNN) mode: points are sorted by x on the host; each 128-row tile
only scans a contiguous window of W sorted candidates. Exactness is
verified per row on the host (found_min <= margin^2 where margin is the
x-distance to the nearest excluded candidate); rows that fail are
recomputed exactly on the host (rare).
"""

import numpy as np

import concourse.bass as bass
import concourse.mybir as mybir
import concourse.tile as tile
from concourse import bacc
from concourse.bass_utils import run_bass_kernel_spmd

F32 = mybir.dt.float32

N = 16384  # points1 rows
M = 16384  # points2 rows
NCORES = 8
P = 128  # partitions / rows per tile
TILES_PER_DIR = N // NCORES // P  # 16
NT = 2 * TILES_PER_DIR  # 32 row-tiles per core (16 per direction)

# Candidate window per row tile. W == M -> brute force over all candidates.
# 384 chosen from the margin statistics of sorted randn data: 497/32768 rows
# (1.5%) need the exact host fallback (~0.2s numpy). Cost model 36.6us vs
# 128us at W=2048.
WINDOW = 384
CHUNK = 2048  # psum chunk: 4 banks of 512 fp32
# Packed layout: 4 tiles per group at 32-aligned partition strips
# (tile_position row groups), padded rhs -> full-width [128, W] DMAs.
PACKED = True


def _build_nc_packed(window: int, chunk: int = CHUNK, repeats: int = 1):
    """Row-packed variant: 4 row-tiles share the PE array via tile_position
    row groups (K=5 uses a 32-row strip; 4 strips run concurrently).

    Inputs (per core):
      lhs_aug [NG, 128, 128]   group g, partitions 32i..32i+4 = lhsT of tile 4g+i
                               (pad partitions are zero -> contribute nothing)
      rhs_win [NT, 5, window]  augmented candidate windows (rhs form, compact)
    Output:
      minima  [P, NT]
    """
    assert window % 128 == 0
    n_iters = max(1, (window + 511) // 512)  # 512 cols per tile per iter
    NG = NT // 4
    brute = window >= M

    nc = bacc.Bacc(
        "TRN2",
        target_bir_lowering=False,
        debug=False,
        enable_asserts=True,
        num_devices=NCORES,
    )
    lhs_ap = nc.dram_tensor("lhs_aug", [NG, P, P], F32, kind="ExternalInput").ap()
    # rhs padded to 128 partitions: group g rows 32i..32i+4 hold the window
    # of tile 4g+i (zeros elsewhere) -> one full-width DMA per group.
    rhs_shape = [2, 5, M] if brute else [NG, P, window]
    rhs_ap = nc.dram_tensor("rhs_win", rhs_shape, F32, kind="ExternalInput").ap()
    # repeats>1 is a timing variant: widen the output so no repeat is dead
    # code (bacc DCE would elide overwritten work).
    out_ap = nc.dram_tensor(
        "minima", [P, NT * repeats], F32, kind="ExternalOutput"
    ).ap()

    with tile.TileContext(nc) as tc:
        with (
            tc.tile_pool(name="lhs", bufs=3) as lhs_pool,
            tc.tile_pool(name="rhs", bufs=1 if brute else 3) as rhs_pool,
            tc.tile_pool(name="psum", bufs=2, space="PSUM") as psum_pool,
            tc.tile_pool(name="stage", bufs=3) as stage_pool,
            tc.tile_pool(name="outp", bufs=1) as out_pool,
        ):
            if brute:
                rhs_full = []
                for d in range(2):
                    rf = rhs_pool.tile([P, M], F32, tag=f"rhs{d}")
                    for i in range(4):
                        nc.sync.dma_start(rf[32 * i : 32 * i + 5, :], rhs_ap[d])
                    rhs_full.append(rf)
            out_sb = out_pool.tile([P, NT * repeats], F32)
            for rep in range(repeats):
              for g in range(NG):
                lhs_sb = lhs_pool.tile([P, P], F32)
                nc.sync.dma_start(lhs_sb[:], lhs_ap[g])
                if brute:
                    rhs_sb = rhs_full[0] if g < NG // 2 else rhs_full[1]
                else:
                    rhs_sb = rhs_pool.tile([P, window], F32)
                    nc.sync.dma_start(rhs_sb[:], rhs_ap[g])
                if n_iters > 1:
                    stage = stage_pool.tile([P, 4 * n_iters], F32)
                for it in range(n_iters):
                    col0 = it * 512
                    mw = min(512, window - col0)
                    ps = psum_pool.tile([P, 4 * 512], F32, tag="ps")
                    for i in range(4):
                        nc.tensor.matmul(
                            ps[:, i * 512 : i * 512 + mw],
                            lhs_sb[32 * i : 32 * i + 5, :],
                            rhs_sb[32 * i : 32 * i + 5, col0 : col0 + mw],
                            start=True,
                            stop=True,
                            tile_position=(32 * i, 0),
                        )
                    # one reduce over [P, 4, mw]: bank b = tile 4g+b partial
                    # (slice excludes stale psum columns beyond mw)
                    red_src = ps[:].rearrange("p (t w) -> p t w", t=4)
                    if mw < 512:
                        red_src = red_src[:, :, :mw]
                    dst = (
                        stage[:, it * 4 : (it + 1) * 4]
                        if n_iters > 1
                        else out_sb[:, rep * NT + 4 * g : rep * NT + 4 * (g + 1)]
                    )
                    nc.vector.tensor_reduce(
                        dst,
                        red_src,
                        axis=mybir.AxisListType.X,
                        op=mybir.AluOpType.min,
                    )
                if n_iters > 1:
                    # per tile i: min over chunks -> out_sb[:, 4g+i]
                    nc.vector.tensor_reduce(
                        out_sb[:, rep * NT + 4 * g : rep * NT + 4 * (g + 1)],
                        stage[:].rearrange("p (cb t) -> p t cb", t=4),
                        axis=mybir.AxisListType.X,
                        op=mybir.AluOpType.min,
                    )
            nc.sync.dma_start(out_ap[:], out_sb[:])

    nc.compile()
    return nc


def _build_nc(
    window: int,
    chunk: int = CHUNK,
    repeats: int = 1,
    share_rhs: bool = False,
    reduce_width: int | None = None,
    use_f32r: bool = False,
    batched: bool = True,
):
    """Build the SPMD program (same for all cores).

    Inputs (per core):
      lhs_aug [NT, 5, 128]     augmented row tiles (lhsT form)
      rhs_win [NT, 5, window]  augmented candidate windows (rhs form)
    Output:
      minima  [128, NT]        min d2 per row of each tile
    """
    assert window % 128 == 0 and chunk % 512 == 0
    n_chunks = (window + chunk - 1) // chunk

    brute = window >= M

    nc = bacc.Bacc(
        "TRN2",
        target_bir_lowering=False,
        debug=False,
        enable_asserts=True,
        num_devices=NCORES,
    )
    IN_DT = mybir.dt.float32r if use_f32r else F32
    batched = batched and not brute and not share_rhs
    SPAN = 4  # tiles per rhs DMA in batched mode
    if batched:
        lhs_ap = nc.dram_tensor(
            "lhs_aug", [5, NT * P], IN_DT, kind="ExternalInput"
        ).ap()
        rhs_ap = nc.dram_tensor(
            "rhs_win", [NT // SPAN, 5, SPAN * window], IN_DT, kind="ExternalInput"
        ).ap()
    else:
        lhs_ap = nc.dram_tensor(
            "lhs_aug", [NT, 5, P], IN_DT, kind="ExternalInput"
        ).ap()
        rhs_shape = [2, 5, M] if brute else [NT, 5, window]
        rhs_ap = nc.dram_tensor("rhs_win", rhs_shape, IN_DT, kind="ExternalInput").ap()
    out_ap = nc.dram_tensor(
        "minima", [P, NT * repeats], F32, kind="ExternalOutput"
    ).ap()

    with tile.TileContext(nc) as tc:
        with (
            tc.tile_pool(name="lhs", bufs=4) as lhs_pool,
            tc.tile_pool(name="rhs", bufs=1 if brute else 3) as rhs_pool,
            tc.tile_pool(name="psum", bufs=2, space="PSUM") as psum_pool,
            tc.tile_pool(name="stage", bufs=3) as stage_pool,
            tc.tile_pool(name="outp", bufs=1) as out_pool,
        ):
            if brute:
                rhs_full = []
                for d in range(2):
                    rf = rhs_pool.tile([5, M], IN_DT, tag=f"rhs{d}")
                    nc.sync.dma_start(rf[:], rhs_ap[d])
                    rhs_full.append(rf)
            out_sb = out_pool.tile([P, NT * repeats], F32)
            fused = batched and window == 512
            if fused:
                # fused path: 1 MM per tile, 4 tiles share one 4-bank psum
                # tile, one strided reduce -> 4 output columns. No stage.
                for rep in range(repeats):
                    lhs_all = lhs_pool.tile([5, NT * P], IN_DT, tag="lhsall")
                    nc.sync.dma_start(lhs_all[:], lhs_ap[:])
                    for gg in range(NT // SPAN):
                        rhs_span = rhs_pool.tile(
                            [5, SPAN * window], IN_DT, tag="rspan"
                        )
                        nc.sync.dma_start(rhs_span[:], rhs_ap[gg])
                        ps = psum_pool.tile([P, SPAN * 512], F32, tag="ps")
                        for i in range(SPAN):
                            t = gg * SPAN + i
                            nc.tensor.matmul(
                                ps[:, i * 512 : (i + 1) * 512],
                                lhs_all[:, t * P : (t + 1) * P],
                                rhs_span[:, i * window : (i + 1) * window],
                                start=True,
                                stop=True,
                            )
                        nc.vector.tensor_reduce(
                            out_sb[
                                :,
                                rep * NT + gg * SPAN : rep * NT + (gg + 1) * SPAN,
                            ],
                            ps[:].rearrange("p (t w) -> p t w", t=SPAN),
                            axis=mybir.AxisListType.X,
                            op=mybir.AluOpType.min,
                        )
            shared_rhs = None
            for rep in range(repeats if not fused else 0):
              if batched:
                  lhs_all = lhs_pool.tile([5, NT * P], IN_DT, tag="lhsall")
                  nc.sync.dma_start(lhs_all[:], lhs_ap[:])
              rhs_span = None
              for t in range(NT):
                if batched:
                    lhs_sb = lhs_all[:, t * P : (t + 1) * P]
                    if t % SPAN == 0:
                        rhs_span = rhs_pool.tile(
                            [5, SPAN * window], IN_DT, tag="rspan"
                        )
                        nc.sync.dma_start(rhs_span[:], rhs_ap[t // SPAN])
                    rhs_sb = rhs_span[
                        :, (t % SPAN) * window : (t % SPAN + 1) * window
                    ]
                else:
                    lhs_sb = lhs_pool.tile([5, P], IN_DT)
                    nc.sync.dma_start(lhs_sb[:], lhs_ap[t])
                    if brute:
                        rhs_sb = rhs_full[0] if t < TILES_PER_DIR else rhs_full[1]
                    elif share_rhs:
                        # bench-only variant: one rhs window for all tiles
                        if shared_rhs is None:
                            shared_rhs = rhs_pool.tile(
                                [5, window], F32, tag="shared"
                            )
                            nc.sync.dma_start(shared_rhs[:], rhs_ap[0])
                        rhs_sb = shared_rhs
                    else:
                        rhs_sb = rhs_pool.tile([5, window], IN_DT)
                        nc.sync.dma_start(rhs_sb[:], rhs_ap[t])

                if n_chunks > 1:
                    stage = stage_pool.tile([P, n_chunks], F32)
                for cb in range(n_chunks):
                    cw = min(chunk, window - cb * chunk)
                    ps = psum_pool.tile([P, chunk], F32, tag="ps")
                    for k in range(0, cw, 512):
                        mw = min(512, cw - k)
                        nc.tensor.matmul(
                            ps[:, k : k + mw],
                            lhs_sb[:],
                            rhs_sb[:, cb * chunk + k : cb * chunk + k + mw],
                            start=True,
                            stop=True,
                        )
                    oc = rep * NT + t
                    dst = stage[:, cb : cb + 1] if n_chunks > 1 else out_sb[:, oc : oc + 1]
                    rw = cw if reduce_width is None else min(reduce_width, cw)
                    nc.vector.tensor_reduce(
                        dst,
                        ps[:, :rw],
                        axis=mybir.AxisListType.X,
                        op=mybir.AluOpType.min,
                    )
                if n_chunks > 1:
                    nc.vector.tensor_reduce(
                        out_sb[:, oc : oc + 1],
                        stage[:],
                        axis=mybir.AxisListType.X,
                        op=mybir.AluOpType.min,
                    )
            nc.sync.dma_start(out_ap[:], out_sb[:])

    nc.compile()
    return nc


def _rep_tiles(repeats, n=NT):
    for _ in range(repeats):
        yield from range(n)


def _augment(pts):
    """pts [n, 3] f32 -> (A [5, n] lhsT form, B [5, n] rhs form, sq [n])."""
    x = np.ascontiguousarray(pts, dtype=np.float32)
    sq = (x[:, 0] * x[:, 0] + x[:, 1] * x[:, 1] + x[:, 2] * x[:, 2]).astype(
        np.float32
    )
    ones = np.ones_like(sq)
    A = np.stack([x[:, 0], x[:, 1], x[:, 2], sq, ones], axis=0)
    B = np.stack([-2.0 * x[:, 0], -2.0 * x[:, 1], -2.0 * x[:, 2], ones, sq], axis=0)
    return np.ascontiguousarray(A), np.ascontiguousarray(B), sq


_NC_CACHE: dict = {}


def _get_nc(window: int, repeats: int = 1, **variant):
    key = (window, repeats, tuple(sorted(variant.items())))
    nc = _NC_CACHE.get(key)
    if nc is None:
        v = dict(variant)
        if v.pop("packed", PACKED):
            assert not v, f"packed builder has no variants: {v}"
            nc = _build_nc_packed(window, repeats=repeats)
        else:
            nc = _build_nc(window, repeats=repeats, **v)
        _NC_CACHE[key] = nc
    return nc


def _prepare_inputs(
    points1, points2, window: int, packed: bool = PACKED, batched: bool = True
):
    """Host-side shard/window prep. Returns (in_maps, meta) where meta holds
    what's needed to finish/verify on the host."""
    p1 = np.ascontiguousarray(points1, dtype=np.float32)
    p2 = np.ascontiguousarray(points2, dtype=np.float32)

    if window >= M:
        ord1 = np.arange(N)
        ord2 = np.arange(M)
    else:
        ord1 = np.argsort(p1[:, 0], kind="stable")
        ord2 = np.argsort(p2[:, 0], kind="stable")
    s1 = p1[ord1]
    s2 = p2[ord2]
    A1, B1, _ = _augment(s1)
    A2, B2, _ = _augment(s2)

    n_tiles_total = N // P  # 128 row tiles per direction
    # Window start (in sorted candidate ranks) per global row tile.
    if window >= M:
        c0_1 = np.zeros(n_tiles_total, dtype=np.int64)  # p1 tiles scan all p2
        c0_2 = np.zeros(n_tiles_total, dtype=np.int64)
    else:

        def _starts(xs_rows, xs_cands):
            starts = np.empty(n_tiles_total, dtype=np.int64)
            for g in range(n_tiles_total):
                lo = np.searchsorted(xs_cands, xs_rows[g * P])
                hi = np.searchsorted(xs_cands, xs_rows[(g + 1) * P - 1])
                c = (lo + hi) // 2 - window // 2
                starts[g] = min(max(c, 0), len(xs_cands) - window)
            return starts

        c0_1 = _starts(s1[:, 0], s2[:, 0])
        c0_2 = _starts(s2[:, 0], s1[:, 0])

    brute = window >= M
    batched = batched and not packed and not brute
    rhs_brute = np.stack([B2, B1]) if brute else None
    SPAN = 4
    in_maps = []
    for c in range(NCORES):
        if packed:
            lhs = np.zeros((NT // 4, P, P), dtype=np.float32)
        else:
            lhs = np.empty((NT, 5, P), dtype=np.float32)
        rhs = rhs_brute if brute else np.empty((NT, 5, window), dtype=np.float32)
        for tl in range(NT):
            g = c * TILES_PER_DIR + (tl % TILES_PER_DIR)
            asrc = A1 if tl < TILES_PER_DIR else A2
            tile_lhs = asrc[:, g * P : (g + 1) * P]
            if packed:
                lhs[tl // 4, 32 * (tl % 4) : 32 * (tl % 4) + 5, :] = tile_lhs
            else:
                lhs[tl] = tile_lhs
            if not brute:
                csrc = (B2, c0_1) if tl < TILES_PER_DIR else (B1, c0_2)
                rhs[tl] = csrc[0][:, csrc[1][g] : csrc[1][g] + window]
        if packed and not brute:
            # [NT,5,W] -> padded [NT/4, 128, W]: rows 32i..32i+4 = tile 4g+i
            rp = np.zeros((NT // 4, P, window), dtype=np.float32)
            for tl in range(NT):
                rp[tl // 4, 32 * (tl % 4) : 32 * (tl % 4) + 5, :] = rhs[tl]
            rhs = rp
        elif batched:
            # [NT,5,P] -> [5, NT*P]; [NT,5,W] -> [NT/SPAN, 5, SPAN*W]
            lhs = np.ascontiguousarray(
                lhs.transpose(1, 0, 2).reshape(5, NT * P)
            )
            rhs = np.ascontiguousarray(
                rhs.reshape(NT // SPAN, SPAN, 5, window)
                .transpose(0, 2, 1, 3)
                .reshape(NT // SPAN, 5, SPAN * window)
            )
        in_maps.append({"lhs_aug": lhs, "rhs_win": rhs})

    meta = dict(s1=s1, s2=s2, c0_1=c0_1, c0_2=c0_2, window=window)
    return in_maps, meta


def _finish(results, meta):
    """Gather per-core minima, verify window margins, fall back exactly where
    needed, and return the chamfer sum."""
    window = meta["window"]
    s1, s2 = meta["s1"], meta["s2"]
    n_tiles_total = N // P

    # m1[g*P + p] = min d2 for sorted-p1 row of global tile g, partition p.
    m1 = np.empty(N, dtype=np.float32)
    m2 = np.empty(M, dtype=np.float32)
    for c in range(NCORES):
        mins = results[c]["minima"]  # [P, NT]
        for tl in range(TILES_PER_DIR):
            g = c * TILES_PER_DIR + tl
            m1[g * P : (g + 1) * P] = mins[:, tl]
            m2[g * P : (g + 1) * P] = mins[:, TILES_PER_DIR + tl]

    if window < M:

        def _verify_fix(mvals, rows, cands, c0s):
            xs_r = rows[:, 0]
            xs_c = cands[:, 0]
            ncand = len(xs_c)
            starts = np.repeat(c0s, P)
            left = np.where(
                starts > 0, xs_r - xs_c[np.maximum(starts - 1, 0)], np.inf
            )
            ends = starts + window
            right = np.where(
                ends < ncand, xs_c[np.minimum(ends, ncand - 1)] - xs_r, np.inf
            )
            margin = np.minimum(left, right)
            bad = ~(mvals <= (margin * margin))
            nbad = int(bad.sum())
            if nbad:
                d = (
                    rows[bad, None, :].astype(np.float64)
                    - cands[None, :, :].astype(np.float64)
                ) ** 2
                mvals[bad] = d.sum(-1).min(1).astype(np.float32)
            return nbad

        nb1 = _verify_fix(m1, s1, s2, meta["c0_1"])
        nb2 = _verify_fix(m2, s2, s1, meta["c0_2"])
        _finish.fallback_rows = nb1 + nb2
    else:
        _finish.fallback_rows = 0

    total = np.sqrt(np.maximum(m1, 0.0).astype(np.float64)).sum() + np.sqrt(
        np.maximum(m2, 0.0).astype(np.float64)
    ).sum()
    return np.float32(total)


_EXEC_CACHE: dict = {}


def _get_exec(window: int, repeats: int = 1, **variant):
    """Build (once) a persistent jitted shard_map executable for the program.

    Mirrors concourse.bass2jax.run_bass_via_pjrt, but caches the jitted
    callable so repeat calls don't re-trace. `repeats` selects a program
    variant with the whole tile loop unrolled `repeats` times (for timing:
    slope between repeats=R and repeats=1 isolates pure kernel time).
    """
    key = (window, repeats, tuple(sorted(variant.items())))
    if key in _EXEC_CACHE:
        return _EXEC_CACHE[key]

    import jax
    from jax.sharding import Mesh, PartitionSpec
    from jax.experimental.shard_map import shard_map

    from concourse.bass2jax import (
        _bass_exec_p,
        install_neuronx_cc_hook,
        partition_id_tensor,
    )

    nc = _get_nc(window, repeats, **variant)
    install_neuronx_cc_hook()
    assert nc.dbg_addr is None
    partition_name = (
        nc.partition_id_tensor.name if nc.partition_id_tensor is not None else None
    )

    in_names, out_names, out_avals, zero_shapes = [], [], [], []
    for alloc in nc.m.functions[0].allocations:
        if not isinstance(alloc, mybir.MemoryLocationSet):
            continue
        name = alloc.memorylocations[0].name
        if alloc.kind == "ExternalInput":
            if name != partition_name:
                in_names.append(name)
        elif alloc.kind == "ExternalOutput":
            shape = tuple(alloc.tensor_shape)
            dtype = mybir.dt.np(alloc.dtype)
            out_names.append(name)
            out_avals.append(jax.core.ShapedArray(shape, dtype))
            zero_shapes.append((shape, dtype))
    n_params = len(in_names)
    all_names = in_names + out_names
    if partition_name is not None:
        all_names = all_names + [partition_name]
    all_names = tuple(all_names)

    def _body(*args):
        operands = list(args)
        if partition_name is not None:
            operands.append(partition_id_tensor())
        outs = _bass_exec_p.bind(
            *operands,
            out_avals=tuple(out_avals),
            in_names=all_names,
            out_names=tuple(out_names),
            lowering_input_output_aliases=(),
            sim_require_finite=True,
            sim_require_nnan=True,
            nc=nc,
        )
        return tuple(outs)

    devices = jax.devices()[:NCORES]
    mesh = Mesh(np.asarray(devices), ("core",))
    n_outs = len(out_names)
    donate = tuple(range(n_params, n_params + n_outs))

    fn = jax.jit(
        shard_map(
            _body,
            mesh=mesh,
            in_specs=(PartitionSpec("core"),) * (n_params + n_outs),
            out_specs=(PartitionSpec("core"),) * n_outs,
            check_rep=False,
        ),
        donate_argnums=donate,
        keep_unused=True,
    )

    info = dict(
        nc=nc,
        mesh=mesh,
        in_names=in_names,
        out_names=out_names,
        out_avals=out_avals,
        zero_shapes=zero_shapes,
        n_params=n_params,
        fn=fn,
    )
    _EXEC_CACHE[key] = info
    return info


def _concat_inputs(info, in_maps):
    return [
        np.concatenate([np.asarray(m[name]) for m in in_maps], axis=0)
        for name in info["in_names"]
    ]


def _zeros(info):
    return [
        np.zeros((NCORES * s[0], *s[1:]), d) for (s, d) in info["zero_shapes"]
    ]


def _execute(info, concat_in):
    import jax

    out_arrs = jax.block_until_ready(info["fn"](*concat_in, *_zeros(info)))
    return out_arrs


def _split_results(info, out_arrs):
    results = []
    for c in range(NCORES):
        results.append(
            {
                name: np.asarray(out_arrs[i]).reshape(
                    NCORES, *info["out_avals"][i].shape
                )[c]
                for i, name in enumerate(info["out_names"])
            }
        )
    return results


def _run(points1, points2, window=WINDOW, trace=False, **variant):
    info = _get_exec(window, **variant)
    in_maps, meta = _prepare_inputs(
        points1, points2, window, packed=variant.get("packed", PACKED)
    )
    out_arrs = _execute(info, _concat_inputs(info, in_maps))
    results = _split_results(info, out_arrs)
    out = _finish(results, meta)
    return out, results


def _time_async_batch(info, concat_in, ncalls):
    """Launch ncalls back-to-back without blocking, block at the end."""
    import time

    import jax

    outs = None
    t0 = time.perf_counter()
    for _ in range(ncalls):
        outs = info["fn"](*concat_in, *_zeros(info))
    jax.block_until_ready(outs)
    return time.perf_counter() - t0


def _bench_async(points1, points2, window=WINDOW, repeats=5, ncalls=60, reps=3, **variant):
    """Kernel time via async-batch slope between repeats=R and repeats=1
    programs: T = (batchwall_R - batchwall_1) / (ncalls * (R - 1))."""
    import jax
    from jax.sharding import NamedSharding, PartitionSpec

    in_maps, _ = _prepare_inputs(
        points1, points2, window, packed=variant.get("packed", PACKED)
    )
    walls = {}
    for r in (1, repeats):
        info = _get_exec(window, repeats=r, **variant)
        sharding = NamedSharding(info["mesh"], PartitionSpec("core"))
        concat_in = [
            jax.device_put(x, sharding) for x in _concat_inputs(info, in_maps)
        ]
        _time_async_batch(info, concat_in, 3)  # warm
        walls[r] = min(
            _time_async_batch(info, concat_in, ncalls) for _ in range(reps)
        )
    per_exec_ns = (
        (walls[repeats] - walls[1]) / (ncalls * (repeats - 1)) * 1e9
    )
    return per_exec_ns, walls


def _time_exec(info, concat_in, reps):
    import time

    best = float("inf")
    for _ in range(reps):
        t0 = time.perf_counter()
        _execute(info, concat_in)
        best = min(best, time.perf_counter() - t0)
    return best


def _bench(points1, points2, window=WINDOW, repeats=5, reps=5, **variant):
    """Per-execution kernel time (ns) via the repeated-program slope:
    T = (wall(program x R) - wall(program x 1)) / (R - 1)."""
    import jax
    from jax.sharding import NamedSharding, PartitionSpec

    in_maps, _ = _prepare_inputs(
        points1, points2, window, packed=variant.get("packed", PACKED)
    )

    walls = {}
    for r in (1, repeats):
        info = _get_exec(window, repeats=r, **variant)
        sharding = NamedSharding(info["mesh"], PartitionSpec("core"))
        concat_in = [
            jax.device_put(x, sharding) for x in _concat_inputs(info, in_maps)
        ]
        _execute(info, concat_in)  # warm
        walls[r] = _time_exec(info, concat_in, reps)

    per_exec_ns = (walls[repeats] - walls[1]) / (repeats - 1) * 1e9
    return per_exec_ns, walls[1] * 1e9


def _host_reference(points1, points2):
    """Pure-numpy fallback (same fp32 expansion math), used only if the
    device path fails."""
    p1 = np.ascontiguousarray(points1, dtype=np.float32)
    p2 = np.ascontiguousarray(points2, dtype=np.float32)
    A1, B1, _ = _augment(p1)
    A2, B2, _ = _augment(p2)
    total = 0.0
    for A, Bo in ((A1, B2), (A2, B1)):
        mins = np.empty(A.shape[1], dtype=np.float32)
        for i in range(0, A.shape[1], 2048):
            d2 = A[:, i : i + 2048].T @ Bo  # fp32 BLAS
            mins[i : i + 2048] = d2.min(axis=1)
        total += np.sqrt(np.maximum(mins, 0.0).astype(np.float64)).sum()
    return np.float32(total)


def kernel(points1, points2):
    try:
        out, _ = _run(points1, points2)
        return out
    except Exception:
        import traceback

        traceback.print_exc()
        return _host_reference(points1, points2)



# revision 3
# speedup vs baseline: 40.7624x; 40.7624x over previous
"""Chamfer distance kernel for 8 TRN2 NeuronCores (Bass/Tile).

Problem: points1 [16384, 3], points2 [16384, 3] fp32.
  out = sum_i min_j ||p1_i - p2_j|| + sum_j min_i ||p1_i - p2_j||   (scalar)

Strategy
--------
sqrt is monotonic, so min_j ||.|| = sqrt(min_j ||.||^2): only squared
distances are ever materialized, and sqrt runs on the host over the 32K
row-minima.

Squared distances come from a single K=5 matmul with augmented points:
    a_i = [x, y, z, |p|^2, 1]          (lhsT form, stationary)
    b_j = [-2x, -2y, -2z, 1, |p|^2]    (rhs form, moving)
    a_i . b_j = |p1_i|^2 + |p2_j|^2 - 2 p1_i.p2_j = d2(i, j)

Both chamfer terms are row-min problems (term2 is a row-min of the
transposed matrix = distance matrix with roles swapped), so each core
computes row-minima for 2048 rows of D(p1_chunk, p2) and 2048 rows of
D(p2_chunk, p1): 32 row-tiles of 128 rows. Row-min over the free axis is a
native DVE tensor_reduce straight out of PSUM - no partition-axis
reductions, no collectives.

Windowed (KNN) mode: points are sorted by x on the host; each 128-row tile
only scans a contiguous window of W sorted candidates. Exactness is
verified per row on the host (found_min <= margin^2 where margin is the
x-distance to the nearest excluded candidate); rows that fail are
recomputed exactly on the host (rare: ~1.5% at W=384).

Device program (v2, per core)
-----------------------------
All data is SBUF-resident up front via 8 compact strip DMAs (~320 KB):
strip q in [0,4) lives at partitions 32q..32q+5, holding the lhsT/rhs of
tiles t = 4g+q for all 8 groups g side by side in the free axis.  Each
group g then runs 4 concurrent matmuls (tile_position row strips share
the PE array) into one 4-bank PSUM tile, and a single tensor_reduce pulls
the 4 row-min columns out.  2 PSUM tiles double-buffer; one [128, 32]
DMA returns all minima.
"""

import numpy as np

import concourse.bass as bass
import concourse.mybir as mybir
import concourse.tile as tile
from concourse import bacc
from concourse.bass_utils import run_bass_kernel_spmd  # noqa: F401 (API ref)

F32 = mybir.dt.float32

N = 16384  # points1 rows
M = 16384  # points2 rows
NCORES = 8
P = 128  # partitions / rows per tile
TILES_PER_DIR = N // NCORES // P  # 16
NT = 2 * TILES_PER_DIR  # 32 row-tiles per core (16 per direction)
NG = NT // 4  # 8 groups of 4 strip-packed tiles

# Candidate window per row tile. Margin statistics of sorted randn data:
# W=384 -> 497/32768 rows (1.5%) host fallback; W=256 -> 4.8%.
WINDOW = 384
# f32r (1 cy/row vs fp32's 4 at N>=256) is numerically unusable here: the
# reduced-precision multiply breaks the |a|^2+|b|^2-2ab cancellation and
# wipes out the small minima (rel err 0.44 measured on HW).
USE_F32R = False


def _build_nc_v2(window: int, repeats: int = 1, use_f32r: bool = USE_F32R,
                 split_dma: int = 1):
    """Strip-resident packed program.

    Inputs (per core):
      lhs_aug [4, 5, NG*128]   strip q row a -> lhsT row a of tiles 4g+q
      rhs_win [4, 5, NG*W]     strip q row a -> rhs row a of tiles 4g+q
    Output:
      minima  [128, NT*repeats]
    """
    assert window <= 512
    IN_DT = mybir.dt.float32r if use_f32r else F32

    nc = bacc.Bacc(
        "TRN2",
        target_bir_lowering=False,
        debug=False,
        enable_asserts=False,
        num_devices=NCORES,
    )
    lhs_ap = nc.dram_tensor("lhs_aug", [4, 5, NG * P], IN_DT, kind="ExternalInput").ap()
    rhs_ap = nc.dram_tensor("rhs_win", [4, 5, NG * window], IN_DT, kind="ExternalInput").ap()
    out_ap = nc.dram_tensor(
        "minima", [P, NT * repeats], F32, kind="ExternalOutput"
    ).ap()

    with tile.TileContext(nc) as tc:
        with (
            tc.tile_pool(name="data", bufs=1) as data_pool,
            tc.tile_pool(name="psum", bufs=2, space="PSUM") as psum_pool,
            tc.tile_pool(name="outp", bufs=1) as out_pool,
        ):
            lhs_sb = data_pool.tile([P, NG * P], IN_DT, tag="lhs")
            rhs_sb = data_pool.tile([P, NG * window], IN_DT, tag="rhs")
            for q in range(4):
                nc.sync.dma_start(lhs_sb[32 * q : 32 * q + 5, :], lhs_ap[q])
                if split_dma == 1:
                    nc.sync.dma_start(rhs_sb[32 * q : 32 * q + 5, :], rhs_ap[q])
                else:
                    step = NG // split_dma
                    for h in range(split_dma):
                        c0 = h * step * window
                        c1 = (h + 1) * step * window
                        nc.sync.dma_start(
                            rhs_sb[32 * q : 32 * q + 5, c0:c1],
                            rhs_ap[q, :, c0:c1],
                        )
            out_sb = out_pool.tile([P, NT * repeats], F32)
            for rep in range(repeats):
                for g in range(NG):
                    ps = psum_pool.tile([P, 4 * 512], F32, tag="ps")
                    for q in range(4):
                        nc.tensor.matmul(
                            ps[:, q * 512 : q * 512 + window],
                            lhs_sb[32 * q : 32 * q + 5, g * P : (g + 1) * P],
                            rhs_sb[32 * q : 32 * q + 5, g * window : (g + 1) * window],
                            start=True,
                            stop=True,
                            tile_position=(32 * q, 0),
                        )
                    red_src = ps[:].rearrange("p (t w) -> p t w", t=4)
                    if window < 512:
                        red_src = red_src[:, :, :window]
                    nc.vector.tensor_reduce(
                        out_sb[:, rep * NT + 4 * g : rep * NT + 4 * (g + 1)],
                        red_src,
                        axis=mybir.AxisListType.X,
                        op=mybir.AluOpType.min,
                    )
            nc.sync.dma_start(out_ap[:], out_sb[:])

    nc.compile()
    return nc


def _augment(pts):
    """pts [n, 3] f32 -> (A [5, n] lhsT form, B [5, n] rhs form, sq [n])."""
    x = np.ascontiguousarray(pts, dtype=np.float32)
    sq = (x[:, 0] * x[:, 0] + x[:, 1] * x[:, 1] + x[:, 2] * x[:, 2]).astype(
        np.float32
    )
    ones = np.ones_like(sq)
    A = np.stack([x[:, 0], x[:, 1], x[:, 2], sq, ones], axis=0)
    B = np.stack([-2.0 * x[:, 0], -2.0 * x[:, 1], -2.0 * x[:, 2], ones, sq], axis=0)
    return np.ascontiguousarray(A), np.ascontiguousarray(B), sq


_NC_CACHE: dict = {}


def _get_nc(window: int, repeats: int = 1, **variant):
    key = (window, repeats, tuple(sorted(variant.items())))
    nc = _NC_CACHE.get(key)
    if nc is None:
        nc = _build_nc_v2(window, repeats=repeats, **variant)
        _NC_CACHE[key] = nc
    return nc


def _prepare_inputs(points1, points2, window: int):
    """Host-side shard/window prep. Returns (in_maps, meta)."""
    p1 = np.ascontiguousarray(points1, dtype=np.float32)
    p2 = np.ascontiguousarray(points2, dtype=np.float32)

    ord1 = np.argsort(p1[:, 0], kind="stable")
    ord2 = np.argsort(p2[:, 0], kind="stable")
    s1 = p1[ord1]
    s2 = p2[ord2]
    A1, B1, _ = _augment(s1)
    A2, B2, _ = _augment(s2)

    n_tiles_total = N // P  # 128 row tiles per direction

    def _starts(xs_rows, xs_cands):
        starts = np.empty(n_tiles_total, dtype=np.int64)
        for g in range(n_tiles_total):
            lo = np.searchsorted(xs_cands, xs_rows[g * P])
            hi = np.searchsorted(xs_cands, xs_rows[(g + 1) * P - 1])
            c = (lo + hi) // 2 - window // 2
            starts[g] = min(max(c, 0), len(xs_cands) - window)
        return starts

    c0_1 = _starts(s1[:, 0], s2[:, 0])
    c0_2 = _starts(s2[:, 0], s1[:, 0])

    in_maps = []
    for c in range(NCORES):
        lhs = np.empty((4, 5, NG, P), dtype=np.float32)
        rhs = np.empty((4, 5, NG, window), dtype=np.float32)
        for tl in range(NT):
            g, q = tl // 4, tl % 4
            gt = c * TILES_PER_DIR + (tl % TILES_PER_DIR)
            asrc = A1 if tl < TILES_PER_DIR else A2
            bsrc, starts = (B2, c0_1) if tl < TILES_PER_DIR else (B1, c0_2)
            lhs[q, :, g, :] = asrc[:, gt * P : (gt + 1) * P]
            s0 = starts[gt]
            rhs[q, :, g, :] = bsrc[:, s0 : s0 + window]
        in_maps.append(
            {
                "lhs_aug": np.ascontiguousarray(lhs.reshape(4, 5, NG * P)),
                "rhs_win": np.ascontiguousarray(rhs.reshape(4, 5, NG * window)),
            }
        )

    meta = dict(s1=s1, s2=s2, c0_1=c0_1, c0_2=c0_2, window=window)
    return in_maps, meta


def _finish(results, meta):
    """Gather per-core minima, verify window margins, fall back exactly where
    needed, and return the chamfer sum."""
    window = meta["window"]
    s1, s2 = meta["s1"], meta["s2"]

    # m1[g*P + p] = min d2 for sorted-p1 row of global tile g, partition p.
    m1 = np.empty(N, dtype=np.float32)
    m2 = np.empty(M, dtype=np.float32)
    for c in range(NCORES):
        mins = results[c]["minima"]  # [P, NT]
        for tl in range(TILES_PER_DIR):
            g = c * TILES_PER_DIR + tl
            m1[g * P : (g + 1) * P] = mins[:, tl]
            m2[g * P : (g + 1) * P] = mins[:, TILES_PER_DIR + tl]

    def _verify_fix(mvals, rows, cands, c0s):
        xs_r = rows[:, 0]
        xs_c = cands[:, 0]
        ncand = len(xs_c)
        starts = np.repeat(c0s, P)
        left = np.where(
            starts > 0, xs_r - xs_c[np.maximum(starts - 1, 0)], np.inf
        )
        ends = starts + window
        right = np.where(
            ends < ncand, xs_c[np.minimum(ends, ncand - 1)] - xs_r, np.inf
        )
        margin = np.minimum(left, right)
        bad = ~(mvals <= (margin * margin))
        nbad = int(bad.sum())
        if nbad:
            # exact fp64 re-scan of the failed rows, chunked to bound temps
            bidx = np.nonzero(bad)[0]
            cd = cands.astype(np.float64)
            for i0 in range(0, nbad, 256):
                sel = bidx[i0 : i0 + 256]
                d = (rows[sel, None, :].astype(np.float64) - cd[None, :, :]) ** 2
                mvals[sel] = d.sum(-1).min(1).astype(np.float32)
        return nbad

    nb1 = _verify_fix(m1, s1, s2, meta["c0_1"])
    nb2 = _verify_fix(m2, s2, s1, meta["c0_2"])
    _finish.fallback_rows = nb1 + nb2

    total = np.sqrt(np.maximum(m1, 0.0).astype(np.float64)).sum() + np.sqrt(
        np.maximum(m2, 0.0).astype(np.float64)
    ).sum()
    return np.float32(total)


_EXEC_CACHE: dict = {}


def _get_exec(window: int, repeats: int = 1, **variant):
    """Build (once) a persistent jitted shard_map executable for the program."""
    key = (window, repeats, tuple(sorted(variant.items())))
    if key in _EXEC_CACHE:
        return _EXEC_CACHE[key]

    import jax
    from jax.sharding import Mesh, PartitionSpec
    from jax.experimental.shard_map import shard_map

    from concourse.bass2jax import (
        _bass_exec_p,
        install_neuronx_cc_hook,
        partition_id_tensor,
    )

    nc = _get_nc(window, repeats, **variant)
    install_neuronx_cc_hook()
    assert nc.dbg_addr is None
    partition_name = (
        nc.partition_id_tensor.name if nc.partition_id_tensor is not None else None
    )

    in_names, out_names, out_avals, zero_shapes = [], [], [], []
    for alloc in nc.m.functions[0].allocations:
        if not isinstance(alloc, mybir.MemoryLocationSet):
            continue
        name = alloc.memorylocations[0].name
        if alloc.kind == "ExternalInput":
            if name != partition_name:
                in_names.append(name)
        elif alloc.kind == "ExternalOutput":
            shape = tuple(alloc.tensor_shape)
            dtype = mybir.dt.np(alloc.dtype)
            out_names.append(name)
            out_avals.append(jax.core.ShapedArray(shape, dtype))
            zero_shapes.append((shape, dtype))
    n_params = len(in_names)
    all_names = in_names + out_names
    if partition_name is not None:
        all_names = all_names + [partition_name]
    all_names = tuple(all_names)

    def _body(*args):
        operands = list(args)
        if partition_name is not None:
            operands.append(partition_id_tensor())
        outs = _bass_exec_p.bind(
            *operands,
            out_avals=tuple(out_avals),
            in_names=all_names,
            out_names=tuple(out_names),
            lowering_input_output_aliases=(),
            sim_require_finite=True,
            sim_require_nnan=True,
            nc=nc,
        )
        return tuple(outs)

    devices = jax.devices()[:NCORES]
    mesh = Mesh(np.asarray(devices), ("core",))
    n_outs = len(out_names)
    donate = tuple(range(n_params, n_params + n_outs))

    fn = jax.jit(
        shard_map(
            _body,
            mesh=mesh,
            in_specs=(PartitionSpec("core"),) * (n_params + n_outs),
            out_specs=(PartitionSpec("core"),) * n_outs,
            check_rep=False,
        ),
        donate_argnums=donate,
        keep_unused=True,
    )

    info = dict(
        nc=nc,
        mesh=mesh,
        in_names=in_names,
        out_names=out_names,
        out_avals=out_avals,
        zero_shapes=zero_shapes,
        n_params=n_params,
        fn=fn,
    )
    _EXEC_CACHE[key] = info
    return info


def _concat_inputs(info, in_maps):
    return [
        np.concatenate([np.asarray(m[name]) for m in in_maps], axis=0)
        for name in info["in_names"]
    ]


def _zeros(info):
    return [
        np.zeros((NCORES * s[0], *s[1:]), d) for (s, d) in info["zero_shapes"]
    ]


def _execute(info, concat_in):
    import jax

    out_arrs = jax.block_until_ready(info["fn"](*concat_in, *_zeros(info)))
    return out_arrs


def _split_results(info, out_arrs):
    results = []
    for c in range(NCORES):
        results.append(
            {
                name: np.asarray(out_arrs[i]).reshape(
                    NCORES, *info["out_avals"][i].shape
                )[c]
                for i, name in enumerate(info["out_names"])
            }
        )
    return results


def _run(points1, points2, window=WINDOW, **variant):
    info = _get_exec(window, **variant)
    in_maps, meta = _prepare_inputs(points1, points2, window)
    out_arrs = _execute(info, _concat_inputs(info, in_maps))
    results = _split_results(info, out_arrs)
    out = _finish(results, meta)
    return out, results


def _host_reference(points1, points2):
    """Pure-numpy fallback (same fp32 expansion math), used only if the
    device path fails."""
    p1 = np.ascontiguousarray(points1, dtype=np.float32)
    p2 = np.ascontiguousarray(points2, dtype=np.float32)
    A1, B1, _ = _augment(p1)
    A2, B2, _ = _augment(p2)
    total = 0.0
    for A, Bo in ((A1, B2), (A2, B1)):
        mins = np.empty(A.shape[1], dtype=np.float32)
        for i in range(0, A.shape[1], 2048):
            d2 = A[:, i : i + 2048].T @ Bo  # fp32 BLAS
            mins[i : i + 2048] = d2.min(axis=1)
        total += np.sqrt(np.maximum(mins, 0.0).astype(np.float64)).sum()
    return np.float32(total)


def kernel(points1, points2):
    try:
        out, _ = _run(points1, points2)
        return out
    except Exception:
        import traceback

        traceback.print_exc()
        return _host_reference(points1, points2)


# revision 8
# speedup vs baseline: 50.5519x; 1.2402x over previous
"""Chamfer distance kernel for 8 TRN2 NeuronCores (Bass/Tile).

Problem: points1 [16384, 3], points2 [16384, 3] fp32.
  out = sum_i min_j ||p1_i - p2_j|| + sum_j min_i ||p1_i - p2_j||   (scalar)

Strategy
--------
sqrt is monotonic, so min_j ||.|| = sqrt(min_j ||.||^2): only squared
distances are ever materialized, and sqrt runs on the host over the 32K
row-minima.

Squared distances come from a single K=5 matmul with augmented points:
    a_i = [x, y, z, |p|^2, 1]          (lhsT form, stationary)
    b_j = [-2x, -2y, -2z, 1, |p|^2]    (rhs form, moving)
    a_i . b_j = |p1_i|^2 + |p2_j|^2 - 2 p1_i.p2_j = d2(i, j)

Both chamfer terms are row-min problems (term2 is a row-min of the
transposed matrix = distance matrix with roles swapped), so each core
computes row-minima for 2048 rows of D(p1_chunk, p2) and 2048 rows of
D(p2_chunk, p1): 32 row-tiles of 128 rows. Row-min over the free axis is a
native DVE tensor_reduce straight out of PSUM - no partition-axis
reductions, no collectives.

Windowed (KNN) mode: points are sorted by x on the host; each 128-row tile
only scans a contiguous window of W sorted candidates. Exactness is
verified per row on the host (found_min <= margin^2 where margin is the
x-distance to the nearest excluded candidate); rows that fail are
recomputed exactly on the host (rare: ~1.5% at W=384).

Device program (v2, per core)
-----------------------------
All data is SBUF-resident up front via 8 compact strip DMAs (~320 KB):
strip q in [0,4) lives at partitions 32q..32q+5, holding the lhsT/rhs of
tiles t = 4g+q for all 8 groups g side by side in the free axis.  Each
group g then runs 4 concurrent matmuls (tile_position row strips share
the PE array) into one 4-bank PSUM tile, and a single tensor_reduce pulls
the 4 row-min columns out.  2 PSUM tiles double-buffer; one [128, 32]
DMA returns all minima.
"""

import numpy as np

import concourse.bass as bass
import concourse.mybir as mybir
import concourse.tile as tile
from concourse import bacc
from concourse.bass_utils import run_bass_kernel_spmd  # noqa: F401 (API ref)

F32 = mybir.dt.float32

N = 16384  # points1 rows
M = 16384  # points2 rows
NCORES = 8
P = 128  # partitions / rows per tile
TILES_PER_DIR = N // NCORES // P  # 16
NT = 2 * TILES_PER_DIR  # 32 row-tiles per core (16 per direction)
NG = NT // 4  # 8 groups of 4 strip-packed tiles

# Candidate window per row tile. Margin statistics of sorted randn data:
# W=384 -> 497/32768 rows (1.5%) host fallback; W=256 -> 4.8%.
WINDOW = 256
# f32r (1 cy/row vs fp32's 4 at N>=256) is numerically unusable here: the
# reduced-precision multiply breaks the |a|^2+|b|^2-2ab cancellation and
# wipes out the small minima (rel err 0.44 measured on HW).
USE_F32R = False


def _build_nc_v2(window: int, repeats: int = 1, use_f32r: bool = USE_F32R,
                 split_dma: int = 1):
    """Strip-resident packed program.

    Inputs (per core):
      lhs_aug [4, 5, NG*128]   strip q row a -> lhsT row a of tiles 4g+q
      rhs_win [4, 5, NG*W]     strip q row a -> rhs row a of tiles 4g+q
    Output:
      minima  [128, NT*repeats]
    """
    assert window <= 512
    IN_DT = mybir.dt.float32r if use_f32r else F32

    nc = bacc.Bacc(
        "TRN2",
        target_bir_lowering=False,
        debug=False,
        enable_asserts=False,
        num_devices=NCORES,
    )
    lhs_ap = nc.dram_tensor("lhs_aug", [4, 5, NG * P], IN_DT, kind="ExternalInput").ap()
    rhs_ap = nc.dram_tensor("rhs_win", [4, 5, NG * window], IN_DT, kind="ExternalInput").ap()
    out_ap = nc.dram_tensor(
        "minima", [P, NT * repeats], F32, kind="ExternalOutput"
    ).ap()

    with tile.TileContext(nc) as tc:
        with (
            tc.tile_pool(name="data", bufs=1) as data_pool,
            tc.tile_pool(name="psum", bufs=2, space="PSUM") as psum_pool,
            tc.tile_pool(name="outp", bufs=1) as out_pool,
        ):
            lhs_sb = data_pool.tile([P, NG * P], IN_DT, tag="lhs")
            rhs_sb = data_pool.tile([P, NG * window], IN_DT, tag="rhs")
            # Trigger engines round-robin: a DMA trigger costs ~700ns on its
            # issuing queue, so spread them instead of serializing on Sync.
            trig = [nc.sync, nc.scalar, nc.gpsimd, nc.sync]
            # First wave: lhs strips (small) + first-half rhs strips so the
            # early groups' matmuls can start while the tail streams in.
            hw = (NG // 2) * window
            for q in range(4):
                trig[q].dma_start(lhs_sb[32 * q : 32 * q + 5, :], lhs_ap[q])
            for q in range(4):
                trig[q].dma_start(
                    rhs_sb[32 * q : 32 * q + 5, :hw], rhs_ap[q, :, :hw]
                )
            for q in range(4):
                trig[q].dma_start(
                    rhs_sb[32 * q : 32 * q + 5, hw:], rhs_ap[q, :, hw:]
                )
            out_sb = out_pool.tile([P, NT * repeats], F32)
            for rep in range(repeats):
                for g in range(NG):
                    ps = psum_pool.tile([P, 4 * 512], F32, tag="ps")
                    for q in range(4):
                        nc.tensor.matmul(
                            ps[:, q * 512 : q * 512 + window],
                            lhs_sb[32 * q : 32 * q + 5, g * P : (g + 1) * P],
                            rhs_sb[32 * q : 32 * q + 5, g * window : (g + 1) * window],
                            start=True,
                            stop=True,
                            tile_position=(32 * q, 0),
                        )
                    red_src = ps[:].rearrange("p (t w) -> p t w", t=4)
                    if window < 512:
                        red_src = red_src[:, :, :window]
                    nc.vector.tensor_reduce(
                        out_sb[:, rep * NT + 4 * g : rep * NT + 4 * (g + 1)],
                        red_src,
                        axis=mybir.AxisListType.X,
                        op=mybir.AluOpType.min,
                    )
            nc.sync.dma_start(out_ap[:], out_sb[:])

    nc.compile()
    return nc


def _augment(pts):
    """pts [n, 3] f32 -> (A [5, n] lhsT form, B [5, n] rhs form, sq [n])."""
    x = np.ascontiguousarray(pts, dtype=np.float32)
    sq = (x[:, 0] * x[:, 0] + x[:, 1] * x[:, 1] + x[:, 2] * x[:, 2]).astype(
        np.float32
    )
    ones = np.ones_like(sq)
    A = np.stack([x[:, 0], x[:, 1], x[:, 2], sq, ones], axis=0)
    B = np.stack([-2.0 * x[:, 0], -2.0 * x[:, 1], -2.0 * x[:, 2], ones, sq], axis=0)
    return np.ascontiguousarray(A), np.ascontiguousarray(B), sq


_NC_CACHE: dict = {}


def _get_nc(window: int, repeats: int = 1, **variant):
    key = (window, repeats, tuple(sorted(variant.items())))
    nc = _NC_CACHE.get(key)
    if nc is None:
        nc = _build_nc_v2(window, repeats=repeats, **variant)
        _NC_CACHE[key] = nc
    return nc


def _prepare_inputs(points1, points2, window: int):
    """Host-side shard/window prep. Returns (in_maps, meta)."""
    p1 = np.ascontiguousarray(points1, dtype=np.float32)
    p2 = np.ascontiguousarray(points2, dtype=np.float32)

    ord1 = np.argsort(p1[:, 0], kind="stable")
    ord2 = np.argsort(p2[:, 0], kind="stable")
    s1 = p1[ord1]
    s2 = p2[ord2]
    A1, B1, _ = _augment(s1)
    A2, B2, _ = _augment(s2)

    n_tiles_total = N // P  # 128 row tiles per direction

    def _starts(xs_rows, xs_cands):
        starts = np.empty(n_tiles_total, dtype=np.int64)
        for g in range(n_tiles_total):
            lo = np.searchsorted(xs_cands, xs_rows[g * P])
            hi = np.searchsorted(xs_cands, xs_rows[(g + 1) * P - 1])
            c = (lo + hi) // 2 - window // 2
            starts[g] = min(max(c, 0), len(xs_cands) - window)
        return starts

    c0_1 = _starts(s1[:, 0], s2[:, 0])
    c0_2 = _starts(s2[:, 0], s1[:, 0])

    in_maps = []
    for c in range(NCORES):
        lhs = np.empty((4, 5, NG, P), dtype=np.float32)
        rhs = np.empty((4, 5, NG, window), dtype=np.float32)
        for tl in range(NT):
            g, q = tl // 4, tl % 4
            gt = c * TILES_PER_DIR + (tl % TILES_PER_DIR)
            asrc = A1 if tl < TILES_PER_DIR else A2
            bsrc, starts = (B2, c0_1) if tl < TILES_PER_DIR else (B1, c0_2)
            lhs[q, :, g, :] = asrc[:, gt * P : (gt + 1) * P]
            s0 = starts[gt]
            rhs[q, :, g, :] = bsrc[:, s0 : s0 + window]
        in_maps.append(
            {
                "lhs_aug": np.ascontiguousarray(lhs.reshape(4, 5, NG * P)),
                "rhs_win": np.ascontiguousarray(rhs.reshape(4, 5, NG * window)),
            }
        )

    meta = dict(s1=s1, s2=s2, c0_1=c0_1, c0_2=c0_2, window=window)
    return in_maps, meta


def _finish(results, meta):
    """Gather per-core minima, verify window margins, fall back exactly where
    needed, and return the chamfer sum."""
    window = meta["window"]
    s1, s2 = meta["s1"], meta["s2"]

    # m1[g*P + p] = min d2 for sorted-p1 row of global tile g, partition p.
    m1 = np.empty(N, dtype=np.float32)
    m2 = np.empty(M, dtype=np.float32)
    for c in range(NCORES):
        mins = results[c]["minima"]  # [P, NT]
        for tl in range(TILES_PER_DIR):
            g = c * TILES_PER_DIR + tl
            m1[g * P : (g + 1) * P] = mins[:, tl]
            m2[g * P : (g + 1) * P] = mins[:, TILES_PER_DIR + tl]

    def _verify_fix(mvals, rows, cands, c0s):
        xs_r = rows[:, 0]
        xs_c = cands[:, 0]
        ncand = len(xs_c)
        starts = np.repeat(c0s, P)
        left = np.where(
            starts > 0, xs_r - xs_c[np.maximum(starts - 1, 0)], np.inf
        )
        ends = starts + window
        right = np.where(
            ends < ncand, xs_c[np.minimum(ends, ncand - 1)] - xs_r, np.inf
        )
        margin = np.minimum(left, right)
        bad = ~(mvals <= (margin * margin))
        nbad = int(bad.sum())
        if nbad:
            # exact fp64 re-scan of the failed rows, chunked to bound temps
            bidx = np.nonzero(bad)[0]
            cd = cands.astype(np.float64)
            for i0 in range(0, nbad, 256):
                sel = bidx[i0 : i0 + 256]
                d = (rows[sel, None, :].astype(np.float64) - cd[None, :, :]) ** 2
                mvals[sel] = d.sum(-1).min(1).astype(np.float32)
        return nbad

    nb1 = _verify_fix(m1, s1, s2, meta["c0_1"])
    nb2 = _verify_fix(m2, s2, s1, meta["c0_2"])
    _finish.fallback_rows = nb1 + nb2

    total = np.sqrt(np.maximum(m1, 0.0).astype(np.float64)).sum() + np.sqrt(
        np.maximum(m2, 0.0).astype(np.float64)
    ).sum()
    return np.float32(total)


_EXEC_CACHE: dict = {}


def _get_exec(window: int, repeats: int = 1, **variant):
    """Build (once) a persistent jitted shard_map executable for the program."""
    key = (window, repeats, tuple(sorted(variant.items())))
    if key in _EXEC_CACHE:
        return _EXEC_CACHE[key]

    import jax
    from jax.sharding import Mesh, PartitionSpec
    from jax.experimental.shard_map import shard_map

    from concourse.bass2jax import (
        _bass_exec_p,
        install_neuronx_cc_hook,
        partition_id_tensor,
    )

    nc = _get_nc(window, repeats, **variant)
    install_neuronx_cc_hook()
    assert nc.dbg_addr is None
    partition_name = (
        nc.partition_id_tensor.name if nc.partition_id_tensor is not None else None
    )

    in_names, out_names, out_avals, zero_shapes = [], [], [], []
    for alloc in nc.m.functions[0].allocations:
        if not isinstance(alloc, mybir.MemoryLocationSet):
            continue
        name = alloc.memorylocations[0].name
        if alloc.kind == "ExternalInput":
            if name != partition_name:
                in_names.append(name)
        elif alloc.kind == "ExternalOutput":
            shape = tuple(alloc.tensor_shape)
            dtype = mybir.dt.np(alloc.dtype)
            out_names.append(name)
            out_avals.append(jax.core.ShapedArray(shape, dtype))
            zero_shapes.append((shape, dtype))
    n_params = len(in_names)
    all_names = in_names + out_names
    if partition_name is not None:
        all_names = all_names + [partition_name]
    all_names = tuple(all_names)

    def _body(*args):
        operands = list(args)
        if partition_name is not None:
            operands.append(partition_id_tensor())
        outs = _bass_exec_p.bind(
            *operands,
            out_avals=tuple(out_avals),
            in_names=all_names,
            out_names=tuple(out_names),
            lowering_input_output_aliases=(),
            sim_require_finite=True,
            sim_require_nnan=True,
            nc=nc,
        )
        return tuple(outs)

    devices = jax.devices()[:NCORES]
    mesh = Mesh(np.asarray(devices), ("core",))
    n_outs = len(out_names)
    donate = tuple(range(n_params, n_params + n_outs))

    fn = jax.jit(
        shard_map(
            _body,
            mesh=mesh,
            in_specs=(PartitionSpec("core"),) * (n_params + n_outs),
            out_specs=(PartitionSpec("core"),) * n_outs,
            check_rep=False,
        ),
        donate_argnums=donate,
        keep_unused=True,
    )

    info = dict(
        nc=nc,
        mesh=mesh,
        in_names=in_names,
        out_names=out_names,
        out_avals=out_avals,
        zero_shapes=zero_shapes,
        n_params=n_params,
        fn=fn,
    )
    _EXEC_CACHE[key] = info
    return info


def _concat_inputs(info, in_maps):
    return [
        np.concatenate([np.asarray(m[name]) for m in in_maps], axis=0)
        for name in info["in_names"]
    ]


def _zeros(info):
    return [
        np.zeros((NCORES * s[0], *s[1:]), d) for (s, d) in info["zero_shapes"]
    ]


def _execute(info, concat_in):
    import jax

    out_arrs = jax.block_until_ready(info["fn"](*concat_in, *_zeros(info)))
    return out_arrs


def _split_results(info, out_arrs):
    results = []
    for c in range(NCORES):
        results.append(
            {
                name: np.asarray(out_arrs[i]).reshape(
                    NCORES, *info["out_avals"][i].shape
                )[c]
                for i, name in enumerate(info["out_names"])
            }
        )
    return results


def _run(points1, points2, window=WINDOW, **variant):
    info = _get_exec(window, **variant)
    in_maps, meta = _prepare_inputs(points1, points2, window)
    out_arrs = _execute(info, _concat_inputs(info, in_maps))
    results = _split_results(info, out_arrs)
    out = _finish(results, meta)
    return out, results


def _host_reference(points1, points2):
    """Pure-numpy fallback (same fp32 expansion math), used only if the
    device path fails."""
    p1 = np.ascontiguousarray(points1, dtype=np.float32)
    p2 = np.ascontiguousarray(points2, dtype=np.float32)
    A1, B1, _ = _augment(p1)
    A2, B2, _ = _augment(p2)
    total = 0.0
    for A, Bo in ((A1, B2), (A2, B1)):
        mins = np.empty(A.shape[1], dtype=np.float32)
        for i in range(0, A.shape[1], 2048):
            d2 = A[:, i : i + 2048].T @ Bo  # fp32 BLAS
            mins[i : i + 2048] = d2.min(axis=1)
        total += np.sqrt(np.maximum(mins, 0.0).astype(np.float64)).sum()
    return np.float32(total)


def kernel(points1, points2):
    try:
        out, _ = _run(points1, points2)
        return out
    except Exception:
        import traceback

        traceback.print_exc()
        return _host_reference(points1, points2)
